# revision 21
# baseline (speedup 1.0000x reference)
"""Trainium2 Bass kernel for the GRU decoder problem.

Math (reference):
    emb[b,t]   = W_emb @ message[b,t] + b_emb                  # [B,T,E]
    xs[t]      = init_emb (t=0) else emb[:, t-1]               # GRU inputs
    gi[t]      = W_ih @ xs[t] + b_ih                           # [B,3H]
    gh         = W_hh @ h + b_hh
    r          = sigmoid(gi_r + gh_r); z = sigmoid(gi_z + gh_z)
    n          = tanh(gi_n + r * gh_n)
    h'         = (1-z)*n + z*h
    out        = sigmoid(W_fc2 @ elu(W_fc1 @ h_T + b_fc1) + b_fc2)

Device strategy (pure data parallel over batch, 8 cores, B/core = 512):
  - The embedding and input-gate projections are fused on host into
    W_combo = W_ih @ W_emb [3H, V]; the per-step input-gate pre-activations
    are computed directly from the fp8 message with fp8 DoubleRow matmuls
    (2 K-subtile pairs instead of 4 bf16 K-chunks), eliminating the separate
    embedding matmul + PSUM eviction of the previous revision.
  - Scaling: message x32 and W_combo x64 keep fp8e4m3 values in the normal
    range; W_hh is pre-scaled by 2048 in bf16 so the PSUM pre-activations are
    uniformly S-scaled, and the 1/S is folded into the ACT scale operand of
    the sigmoid/tanh ops (biases ride along unscaled / pre-scaled).
  - Step 0 of the GRU is batch-independent (h0 = 0, x0 = init_emb), so h1 is
    computed on the host and broadcast; the device scan runs steps 1..63.
  - Batch is split into two 256-wide chains so one chain's serial
    recurrence path hides under the other's engine work. Input matmuls for
    step t+1 are emitted before the gh matmuls of step t so the in-order PE
    queue fills recurrence stalls with independent work.
"""

import numpy as np

import concourse.bass as bass
import concourse.tile as tile
import concourse.mybir as mybir
from concourse.bass_utils import run_bass_kernel_spmd

N_CORES = 8
B, T, V, E, H, FC, O = 4096, 64, 512, 32, 128, 256, 1024
BS = B // N_CORES      # batch per core
TS = T - 1             # message slices consumed by the GRU
NCH = 2                # batch chains per core
CW = BS // NCH         # chain width

SW = 64.0              # host scale on W_combo (fp8 range centering)
SM = 32.0              # host scale on message
S = SW * SM            # combined scale of PSUM pre-activations

F8 = mybir.dt.float8e4
F16 = mybir.dt.float16
BF16 = mybir.dt.bfloat16
F32 = mybir.dt.float32
AF = mybir.ActivationFunctionType
OP = mybir.AluOpType
DR = mybir.MatmulPerfMode.DoubleRow

_PROGRAM = None
LAST_RESULTS = None


# walrus codegen in this toolchain encodes at most 1 sem wait per
# instruction; excess waits are hoisted onto NoOp carriers.
_WAIT_LIMITS: dict = {}
_DEFAULT_WAIT_LIMIT = 1


def _split_excess_waits(nc):
    """Hoist sem waits beyond an instruction encoding's capacity onto
    preceding same-engine NoOp carriers (engines execute their queues in
    order, so waiting earlier on the same engine is equivalent)."""
    for f in nc.m.functions:
        for bb in f.blocks:
            newlist = []
            changed = False
            for inst in bb.instructions:
                si = inst.sync_info
                limit = _WAIT_LIMITS.get(type(inst).__name__, _DEFAULT_WAIT_LIMIT)
                if (
                    limit is not None
                    and si is not None
                    and si.on_wait is not None
                    and len(si.on_wait) > limit
                ):
                    waits = list(si.on_wait)
                    for k, w in enumerate(waits[:-limit]):
                        carrier = mybir.InstNoOp(
                            name=f"{inst.name}-wsplit{k}", ins=[], outs=[]
                        )
                        carrier.engine = inst.engine
                        carrier.sync_info = mybir.SyncInfo(on_wait=[w], on_update=[])
                        newlist.append(carrier)
                    si.on_wait = waits[-limit:]
                    inst.sync_info = si
                    changed = True
                newlist.append(inst)
            if changed:
                bb.instructions[:] = newlist


def _build_program():
    nc = bass.Bass()

    # msg[t, p, k, b] = SM * message[b, t, 128k + p]  (fp8 DoubleRow layout)
    msg = nc.dram_tensor("msg", [TS, 128, 4, BS], F8, kind="ExternalInput")
    # wcombo[p, k, j] = SW * (W_ih @ W_emb)[j, 128k + p]
    wcombod = nc.dram_tensor("wcombo", [128, 4, 3 * H], F8, kind="ExternalInput")
    # S-scaled; sections 3..5 are the negated gate blocks so every gh matmul
    # can be computed as whh@u + (-whh)@v2n from the pre-combine tensors
    # (h' = u - v2n), taking the h' add off the recurrence matmul path.
    whhT = nc.dram_tensor("whhT", [H, 6 * H], BF16, kind="ExternalInput")
    wfc1T = nc.dram_tensor("wfc1T", [H, FC], BF16, kind="ExternalInput")
    wfc2T = nc.dram_tensor("wfc2T", [FC, O], BF16, kind="ExternalInput")
    # bias columns: 0 r, 1 -(z), 2 S*b_in, 3 S*b_hn, 4 h1, 5..6 fc1, 7..14 fc2
    biasd = nc.dram_tensor("bias", [128, 15], F32, kind="ExternalInput")
    out = nc.dram_tensor("out", [O // 128, 128, BS], F32, kind="ExternalOutput")

    with tile.TileContext(nc) as tc:
        with (
            tc.tile_pool(name="const", bufs=1) as const,
            tc.tile_pool(name="msgp", bufs=8) as msgp,
            tc.tile_pool(name="gate", bufs=6) as gate,
            tc.tile_pool(name="fcp", bufs=2) as fcp,
            tc.tile_pool(name="hp", bufs=3) as hp,
            tc.tile_pool(name="outp", bufs=2) as outp,
        ):
            # ---- resident constants ----
            wcombo_sb = const.tile([128, 4, 3 * H], F8)
            nc.sync.dma_start(wcombo_sb[:], wcombod[:])
            whh_sb = const.tile([H, 6 * H], BF16)
            nc.sync.dma_start(whh_sb[:], whhT[:])
            # fc weights ride the (otherwise idle) gpsimd DMA queue so the
            # big wfc2 transfer doesn't delay the message prologue.
            wfc1_sb = const.tile([H, FC], BF16)
            nc.gpsimd.dma_start(wfc1_sb[:], wfc1T[:])
            wfc2_sb = const.tile([128, FC // 128, O], BF16)
            nc.gpsimd.dma_start(wfc2_sb[:],
                                wfc2T.rearrange("(c p) o -> p c o", p=128))
            bias_sb = const.tile([128, 15], F32)
            nc.sync.dma_start(bias_sb[:], biasd[:])
            zeros = const.tile([128, BS], BF16)
            nc.gpsimd.memset(zeros[:], 0.0)

            # h after step 0 is batch-independent (host-computed) -> broadcast.
            # One tile per chain so the chains' recurrences stay decoupled in
            # Tile's dependency tracking.
            h_tiles = {}
            h0c = []
            for c in range(NCH):
                hc = hp.tile([H, CW], BF16, tag="h" + str(c))
                nc.vector.tensor_scalar_add(hc[:], zeros[:, 0:CW],
                                            bias_sb[:, 4:5])
                h0c.append(hc)
            h_tiles[0] = h0c

            # ---- message DMA prologue ----
            LEAD = 6
            msg_tiles = {}

            def dma_msg(j):
                mt = msgp.tile([128, 4, BS], F8)
                nc.sync.dma_start(mt[:], msg[j])
                msg_tiles[j] = mt

            for j in range(min(LEAD, TS)):
                dma_msg(j)

            # Input-gate pre-activations for step st (consumes msg slice
            # st-1): fp8 DoubleRow matmuls, 2 K-subtile pairs per gate, per
            # chain. PSUM dependencies are tile(bank)-granular, so banks are
            # grouped by when their LAST writer lands: pa = [r | gh_n] (all
            # writers gated only on u/v2n), pb = [z | gi_n].
            pa_tiles = {}
            pb_tiles = {}

            with tc.tile_pool(name="psg", bufs=2, space="PSUM") as psg:

                def emit_input_mms(st):
                    mt = msg_tiles[st - 1]
                    pas, pbs = [], []
                    for c in range(NCH):
                        sl = bass.ts(c, CW)
                        pa = psg.tile([128, 2 * CW], F32, tag="pa" + str(c))
                        pb = psg.tile([128, 2 * CW], F32, tag="pb" + str(c))
                        for dst, g, sp_ in (
                            (pa[:, 0:CW], 0, False),
                            (pb[:, 0:CW], 1, False),
                            (pb[:, CW:2 * CW], 2, True),
                        ):
                            for kk in (0, 2):
                                nc.tensor.matmul(
                                    dst,
                                    wcombo_sb[:, kk:kk + 2, bass.ts(g, H)],
                                    mt[:, kk:kk + 2, sl],
                                    start=(kk == 0),
                                    stop=(sp_ and kk == 2),
                                    perf_mode=DR)
                        pas.append(pa)
                        pbs.append(pb)
                    pa_tiles[st] = pas
                    pb_tiles[st] = pbs

                emit_input_mms(1)

                for st in range(1, T):
                    # Input matmuls for step st+1 go to the PE queue before
                    # the gh matmuls of step st: the in-order PE works on
                    # them while waiting for h[st-1].
                    if st + 1 <= T - 1:
                        emit_input_mms(st + 1)

                    hnew_c = []
                    uv_cur = []
                    for c in range(NCH):
                        pa = pa_tiles[st][c]
                        pb = pb_tiles[st][c]
                        hprev = h_tiles[st - 1][c]
                        hnew = hp.tile([H, CW], BF16, tag="h" + str(c))
                        hnew_c.append(hnew)
                        # All gh matmuls come from u/v2n (h' = u - v2n), so
                        # the whole matmul wave is gated only on u; the pa
                        # bank (r, gh_n) is written first so the r sigmoid
                        # releases earliest.
                        if st == 1:
                            nc.tensor.matmul(pa[:, 0:CW], whh_sb[:, 0:H],
                                             hprev[:], start=False, stop=True)
                            nc.tensor.matmul(pa[:, CW:2 * CW],
                                             whh_sb[:, 2 * H:3 * H],
                                             hprev[:], start=True, stop=True)
                            nc.tensor.matmul(pb[:, 0:CW], whh_sb[:, H:2 * H],
                                             hprev[:], start=False, stop=True)
                        else:
                            up, vp = uv_prev[c]
                            for dst, g, st_, sp_ in (
                                (pa[:, 0:CW], 0, False, True),
                                (pa[:, CW:2 * CW], 2, True, True),
                                (pb[:, 0:CW], 1, False, True),
                            ):
                                nc.tensor.matmul(
                                    dst, whh_sb[:, bass.ts(g, H)], up[:],
                                    start=st_, stop=False)
                                nc.tensor.matmul(
                                    dst, whh_sb[:, bass.ts(g + 3, H)], vp[:],
                                    start=False, stop=sp_)

                        r = gate.tile([128, CW], F16, tag="r")
                        nc.scalar.activation(r[:], pa[:, 0:CW], AF.Sigmoid,
                                             bias=bias_sb[:, 0:1],
                                             scale=1.0 / S)
                        # zc = 1 - z = sigmoid(-(x/S + bz)); bias col 1 = -bz
                        zc = gate.tile([128, CW], BF16, tag="zc")
                        nc.scalar.activation(zc[:], pb[:, 0:CW],
                                             AF.Sigmoid,
                                             bias=bias_sb[:, 1:2],
                                             scale=-1.0 / S)
                        # rh = r * (S*gh_n + S*b_hn)
                        rh = gate.tile([128, CW], F16, tag="rh")
                        nc.vector.scalar_tensor_tensor(
                            rh[:], pa[:, CW:2 * CW], bias_sb[:, 3:4], r[:],
                            op0=OP.add, op1=OP.mult)
                        # s = (S*gi_n + S*b_in) + rh
                        s = gate.tile([128, CW], F16, tag="s")
                        nc.vector.scalar_tensor_tensor(
                            s[:], pb[:, CW:2 * CW], bias_sb[:, 2:3], rh[:],
                            op0=OP.add, op1=OP.add)
                        nng = gate.tile([128, CW], BF16, tag="nng")
                        nc.scalar.activation(nng[:], s[:], AF.Tanh,
                                             scale=1.0 / S)
                        # h' = zc*n + (1-zc)*h = zc*n - (zc-1)*h
                        v2n = gate.tile([128, CW], BF16, tag="v2n")
                        nc.vector.scalar_tensor_tensor(
                            v2n[:], zc[:], 1.0, hprev[:],
                            op0=OP.subtract, op1=OP.mult)
                        u = gate.tile([128, CW], BF16, tag="u")
                        nc.vector.tensor_tensor(u[:], nng[:], zc[:],
                                                op=OP.mult)
                        # h' only feeds the next step's v2n and the fc head,
                        # both far off the critical path -> idle gpsimd.
                        nc.gpsimd.tensor_tensor(hnew[:], u[:], v2n[:],
                                                op=OP.subtract)
                        uv_cur.append((u, v2n))
                    h_tiles[st] = hnew_c
                    uv_prev = uv_cur

                    j = LEAD + st - 1
                    if j < TS:
                        dma_msg(j)

            # ---- output head: fc1 + ELU, fc2 + sigmoid ----
            with tc.tile_pool(name="psf", bufs=2, space="PSUM") as psf:
                hlast = h_tiles[T - 1]
                hid = []
                for c in range(FC // 128):
                    pf = psf.tile([128, BS], F32, tag="pf")
                    for ch in range(NCH):
                        nc.tensor.matmul(pf[:, bass.ts(ch, CW)],
                                         wfc1_sb[:, bass.ts(c, 128)],
                                         hlast[ch][:],
                                         start=True, stop=True)
                    bcol = bias_sb[:, 5 + c:6 + c]
                    x1 = fcp.tile([128, BS], BF16, tag="fcx")
                    nc.vector.tensor_scalar_add(x1[:], pf[:], bcol)
                    e1 = fcp.tile([128, BS], F32, tag="fce")
                    nc.scalar.activation(e1[:], pf[:], AF.Exp, bias=bcol)
                    # elu(x) = max(x,0) + min(exp(x)-1, 0)
                    em = fcp.tile([128, BS], BF16, tag="fcm")
                    nc.vector.scalar_tensor_tensor(em[:], e1[:], -1.0,
                                                   zeros[:],
                                                   op0=OP.add, op1=OP.min)
                    hc = fcp.tile([128, BS], BF16, tag="hid" + str(c))
                    nc.vector.scalar_tensor_tensor(hc[:], x1[:], 0.0, em[:],
                                                   op0=OP.max, op1=OP.add)
                    hid.append(hc)
                for o in range(O // 128):
                    po = psf.tile([128, BS], F32, tag="po")
                    for c in range(FC // 128):
                        nc.tensor.matmul(po[:], wfc2_sb[:, c, bass.ts(o, 128)],
                                         hid[c][:], start=(c == 0),
                                         stop=(c == FC // 128 - 1))
                    ob = outp.tile([128, BS], F32)
                    nc.scalar.activation(ob[:], po[:], AF.Sigmoid,
                                         bias=bias_sb[:, 7 + o:8 + o])
                    nc.sync.dma_start(out[o], ob[:])

    _split_excess_waits(nc)
    return nc


def _sigmoid(x):
    return 1.0 / (1.0 + np.exp(-x))


def kernel(message, W_emb, b_emb, init_emb, W_ih, W_hh, b_ih, b_hh,
           W_fc1, b_fc1, W_fc2, b_fc2, _trace=False, _trace_kwargs=None):
    global _PROGRAM, LAST_RESULTS
    if _PROGRAM is None:
        _PROGRAM = _build_program()
    nc = _PROGRAM

    f32 = np.float32

    import ml_dtypes
    bf16 = ml_dtypes.bfloat16
    fp8 = ml_dtypes.float8_e4m3fn

    # message -> per-core fp8 DoubleRow layout [t, p, k, b]; the last token's
    # embedding is never consumed by the GRU so only t = 0..62 is shipped.
    msgT = (
        (message[:, :TS, :] * SM)
        .reshape(N_CORES, BS, TS, 4, 128)
        .transpose(0, 2, 4, 3, 1)  # [core, t, p, k, b]
        .astype(fp8)
    )
    msgT = np.ascontiguousarray(msgT)

    # fused input projection: gi = W_combo @ msg + (W_ih@b_emb + b_ih)
    W_combo = W_ih.astype(np.float64) @ W_emb.astype(np.float64)  # [3H, V]
    wcombo = np.ascontiguousarray(
        (SW * W_combo).T.reshape(4, 128, 3 * H).transpose(1, 0, 2)
    ).astype(fp8)

    whhT3 = (S * W_hh.astype(np.float64)).T  # [H, 3H]
    whhT = np.ascontiguousarray(
        np.concatenate([whhT3, -whhT3], axis=1)
    ).astype(bf16)
    wfc1T = np.ascontiguousarray(W_fc1.T).astype(bf16)
    wfc2T = np.ascontiguousarray(W_fc2.T).astype(bf16)

    b_combo = (W_ih.astype(np.float64) @ b_emb.astype(np.float64)
               + b_ih.astype(np.float64))
    b_hh64 = b_hh.astype(np.float64)

    # step 0 is batch-independent: h0 = 0, x0 = init_emb
    gi0 = W_ih.astype(np.float64) @ init_emb[0, 0].astype(np.float64) + b_ih
    r0 = _sigmoid(gi0[0:H] + b_hh64[0:H])
    z0 = _sigmoid(gi0[H:2 * H] + b_hh64[H:2 * H])
    n0 = np.tanh(gi0[2 * H:3 * H] + r0 * b_hh64[2 * H:3 * H])
    h1 = (1.0 - z0) * n0

    bias = np.zeros((128, 15), f32)
    bias[:, 0] = (b_combo + b_hh64)[0:H]
    bias[:, 1] = -(b_combo + b_hh64)[H:2 * H]
    bias[:, 2] = S * b_combo[2 * H:3 * H]
    bias[:, 3] = S * b_hh64[2 * H:3 * H]
    bias[:, 4] = h1
    bias[:, 5:7] = b_fc1.reshape(2, 128).T.astype(f32)
    bias[:, 7:15] = b_fc2.reshape(8, 128).T.astype(f32)

    shared = dict(wcombo=wcombo, whhT=whhT, wfc1T=wfc1T,
                  wfc2T=wfc2T, bias=bias)
    in_maps = [dict(msg=msgT[c], **shared) for c in range(N_CORES)]

    kw = dict(_trace_kwargs or {})
    res = run_bass_kernel_spmd(nc, in_maps, list(range(N_CORES)),
                               trace=_trace, **kw)
    LAST_RESULTS = res

    outs = [res.results[c]["out"].reshape(O, BS).T for c in range(N_CORES)]
    return np.ascontiguousarray(np.concatenate(outs, axis=0), dtype=f32)


# revision 25
# speedup vs baseline: 1.0758x; 1.0758x over previous
"""Trainium2 Bass kernel for the GRU decoder problem.

Math (reference):
    emb[b,t]   = W_emb @ message[b,t] + b_emb                  # [B,T,E]
    xs[t]      = init_emb (t=0) else emb[:, t-1]               # GRU inputs
    gi[t]      = W_ih @ xs[t] + b_ih                           # [B,3H]
    gh         = W_hh @ h + b_hh
    r          = sigmoid(gi_r + gh_r); z = sigmoid(gi_z + gh_z)
    n          = tanh(gi_n + r * gh_n)
    h'         = (1-z)*n + z*h
    out        = sigmoid(W_fc2 @ elu(W_fc1 @ h_T + b_fc1) + b_fc2)

Device strategy (pure data parallel over batch, 8 cores, B/core = 512):
  - The embedding and input-gate projections are fused on host into
    W_combo = W_ih @ W_emb [3H, V]; the per-step input-gate pre-activations
    are computed directly from the fp8 message with fp8 DoubleRow matmuls
    (2 K-subtile pairs instead of 4 bf16 K-chunks), eliminating the separate
    embedding matmul + PSUM eviction of the previous revision.
  - Scaling: message x32 and W_combo x64 keep fp8e4m3 values in the normal
    range; W_hh is pre-scaled by 2048 in bf16 so the PSUM pre-activations are
    uniformly S-scaled, and the 1/S is folded into the ACT scale operand of
    the sigmoid/tanh ops (biases ride along unscaled / pre-scaled).
  - Step 0 of the GRU is batch-independent (h0 = 0, x0 = init_emb), so h1 is
    computed on the host and broadcast; the device scan runs steps 1..63.
  - Batch is split into two 256-wide chains so one chain's serial
    recurrence path hides under the other's engine work. Input matmuls for
    step t+1 are emitted before the gh matmuls of step t so the in-order PE
    queue fills recurrence stalls with independent work.
"""

import numpy as np

import concourse.bass as bass
import concourse.tile as tile
import concourse.mybir as mybir
from concourse.bass_utils import run_bass_kernel_spmd

N_CORES = 8
B, T, V, E, H, FC, O = 4096, 64, 512, 32, 128, 256, 1024
BS = B // N_CORES      # batch per core
TS = T - 1             # message slices consumed by the GRU
NCH = 2                # batch chains per core
CW = BS // NCH         # chain width

SW = 64.0              # host scale on W_combo (fp8 range centering)
SM = 32.0              # host scale on message
S = SW * SM            # combined scale of PSUM pre-activations

F8 = mybir.dt.float8e4
F16 = mybir.dt.float16
BF16 = mybir.dt.bfloat16
F32 = mybir.dt.float32
AF = mybir.ActivationFunctionType
OP = mybir.AluOpType
DR = mybir.MatmulPerfMode.DoubleRow

_PROGRAM = None
LAST_RESULTS = None


# walrus codegen in this toolchain encodes at most 1 sem wait per
# instruction; excess waits are hoisted onto NoOp carriers.
_WAIT_LIMITS: dict = {}
_DEFAULT_WAIT_LIMIT = 1


def _split_excess_waits(nc):
    """Hoist sem waits beyond an instruction encoding's capacity onto
    preceding same-engine NoOp carriers (engines execute their queues in
    order, so waiting earlier on the same engine is equivalent)."""
    for f in nc.m.functions:
        for bb in f.blocks:
            newlist = []
            changed = False
            for inst in bb.instructions:
                si = inst.sync_info
                limit = _WAIT_LIMITS.get(type(inst).__name__, _DEFAULT_WAIT_LIMIT)
                if (
                    limit is not None
                    and si is not None
                    and si.on_wait is not None
                    and len(si.on_wait) > limit
                ):
                    waits = list(si.on_wait)
                    for k, w in enumerate(waits[:-limit]):
                        carrier = mybir.InstNoOp(
                            name=f"{inst.name}-wsplit{k}", ins=[], outs=[]
                        )
                        carrier.engine = inst.engine
                        carrier.sync_info = mybir.SyncInfo(on_wait=[w], on_update=[])
                        newlist.append(carrier)
                    si.on_wait = waits[-limit:]
                    inst.sync_info = si
                    changed = True
                newlist.append(inst)
            if changed:
                bb.instructions[:] = newlist


def _build_program():
    nc = bass.Bass()

    # msg[t, p, k, b] = SM * message[b, t, 128k + p]  (fp8 DoubleRow layout)
    msg = nc.dram_tensor("msg", [TS, 128, 4, BS], F8, kind="ExternalInput")
    # wcombo[p, k, j] = SW * (W_ih @ W_emb)[j, 128k + p]
    wcombod = nc.dram_tensor("wcombo", [128, 4, 3 * H], F8, kind="ExternalInput")
    # S-scaled; sections 3..5 are the negated gate blocks so every gh matmul
    # can be computed as whh@u + (-whh)@v2n from the pre-combine tensors
    # (h' = u - v2n), taking the h' add off the recurrence matmul path.
    whhT = nc.dram_tensor("whhT", [H, 6 * H], BF16, kind="ExternalInput")
    wfc1T = nc.dram_tensor("wfc1T", [H, FC], BF16, kind="ExternalInput")
    wfc2T = nc.dram_tensor("wfc2T", [FC, O], BF16, kind="ExternalInput")
    # bias columns: 0 r, 1 -(z), 2 S*b_in, 3 S*b_hn, 4 h1, 5..6 fc1, 7..14 fc2
    biasd = nc.dram_tensor("bias", [128, 15], F32, kind="ExternalInput")
    out = nc.dram_tensor("out", [O // 128, 128, BS], F32, kind="ExternalOutput")

    with tile.TileContext(nc) as tc:
        with (
            tc.tile_pool(name="const", bufs=1) as const,
            tc.tile_pool(name="msgp", bufs=8) as msgp,
            tc.tile_pool(name="gate", bufs=6) as gate,
            tc.tile_pool(name="fcp", bufs=2) as fcp,
            tc.tile_pool(name="hp", bufs=3) as hp,
            tc.tile_pool(name="outp", bufs=2) as outp,
        ):
            # ---- resident constants ----
            wcombo_sb = const.tile([128, 4, 3 * H], F8)
            nc.sync.dma_start(wcombo_sb[:], wcombod[:])
            whh_sb = const.tile([H, 6 * H], BF16)
            nc.sync.dma_start(whh_sb[:], whhT[:])
            # fc weights ride the (otherwise idle) gpsimd DMA queue so the
            # big wfc2 transfer doesn't delay the message prologue.
            wfc1_sb = const.tile([H, FC], BF16)
            nc.gpsimd.dma_start(wfc1_sb[:], wfc1T[:])
            wfc2_sb = const.tile([128, FC // 128, O], BF16)
            nc.gpsimd.dma_start(wfc2_sb[:],
                                wfc2T.rearrange("(c p) o -> p c o", p=128))
            bias_sb = const.tile([128, 15], F32)
            nc.sync.dma_start(bias_sb[:], biasd[:])
            zeros = const.tile([128, BS], BF16)
            nc.gpsimd.memset(zeros[:], 0.0)

            # h after step 0 is batch-independent (host-computed) -> broadcast.
            # One tile per chain so the chains' recurrences stay decoupled in
            # Tile's dependency tracking.
            h_tiles = {}
            h0c = []
            for c in range(NCH):
                hc = hp.tile([H, CW], BF16, tag="h" + str(c))
                nc.vector.tensor_scalar_add(hc[:], zeros[:, 0:CW],
                                            bias_sb[:, 4:5])
                h0c.append(hc)
            h_tiles[0] = h0c

            # ---- message DMA prologue ----
            LEAD = 6
            msg_tiles = {}

            def dma_msg(j):
                mt = msgp.tile([128, 4, BS], F8)
                nc.sync.dma_start(mt[:], msg[j])
                msg_tiles[j] = mt

            for j in range(min(LEAD, TS)):
                dma_msg(j)

            # Input-gate pre-activations for step st (consumes msg slice
            # st-1): fp8 DoubleRow matmuls, 2 K-subtile pairs per gate, per
            # chain. PSUM dependencies are tile(bank)-granular, so banks are
            # grouped by when their LAST writer lands: pa = [r | gh_n] (all
            # writers gated only on u/v2n), pb = [z | gi_n].
            pa_tiles = {}
            pb_tiles = {}

            with tc.tile_pool(name="psg", bufs=2, space="PSUM") as psg:

                def emit_input_mms(st):
                    mt = msg_tiles[st - 1]
                    pas, pbs = [], []
                    for c in range(NCH):
                        sl = bass.ts(c, CW)
                        pa = psg.tile([128, 2 * CW], F32, tag="pa" + str(c))
                        pb = psg.tile([128, 2 * CW], F32, tag="pb" + str(c))
                        # pa = [r | gi_n]: every writer is an input matmul or
                        # a u/v2n-bypass matmul, so the r sigmoid never waits
                        # on h'. pb = [z | gh_n]: h'-gated writers, consumed
                        # later (zc, rh) with slack.
                        for dst, g, sp_ in (
                            (pa[:, 0:CW], 0, False),
                            (pb[:, 0:CW], 1, False),
                            (pa[:, CW:2 * CW], 2, True),
                        ):
                            for kk in (0, 2):
                                nc.tensor.matmul(
                                    dst,
                                    wcombo_sb[:, kk:kk + 2, bass.ts(g, H)],
                                    mt[:, kk:kk + 2, sl],
                                    start=(kk == 0),
                                    stop=(sp_ and kk == 2),
                                    perf_mode=DR)
                        pas.append(pa)
                        pbs.append(pb)
                    pa_tiles[st] = pas
                    pb_tiles[st] = pbs

                emit_input_mms(1)

                for st in range(1, T):
                    # Input matmuls for step st+1 go to the PE queue before
                    # the gh matmuls of step st: the in-order PE works on
                    # them while waiting for h[st-1].
                    if st + 1 <= T - 1:
                        emit_input_mms(st + 1)

                    hnew_c = []
                    uv_cur = []
                    for c in range(NCH):
                        pa = pa_tiles[st][c]
                        pb = pb_tiles[st][c]
                        hprev = h_tiles[st - 1][c]
                        hnew = hp.tile([H, CW], BF16, tag="h" + str(c))
                        hnew_c.append(hnew)
                        # gh_r comes from u/v2n (h' = u - v2n) so the r path
                        # skips the h' add; gh_n/gh_z read h' but land in pb,
                        # whose consumers run later.
                        if st == 1:
                            nc.tensor.matmul(pa[:, 0:CW], whh_sb[:, 0:H],
                                             hprev[:], start=False, stop=True)
                        else:
                            up, vp = uv_prev[c]
                            nc.tensor.matmul(pa[:, 0:CW], whh_sb[:, 0:H],
                                             up[:], start=False, stop=False)
                            nc.tensor.matmul(pa[:, 0:CW],
                                             whh_sb[:, 3 * H:4 * H], vp[:],
                                             start=False, stop=True)
                        nc.tensor.matmul(pb[:, CW:2 * CW],
                                         whh_sb[:, 2 * H:3 * H],
                                         hprev[:], start=True, stop=True)
                        nc.tensor.matmul(pb[:, 0:CW], whh_sb[:, H:2 * H],
                                         hprev[:], start=False, stop=True)

                        r = gate.tile([128, CW], F16, tag="r")
                        nc.scalar.activation(r[:], pa[:, 0:CW], AF.Sigmoid,
                                             bias=bias_sb[:, 0:1],
                                             scale=1.0 / S)
                        # zc = 1 - z = sigmoid(-(x/S + bz)); bias col 1 = -bz
                        zc = gate.tile([128, CW], BF16, tag="zc")
                        nc.scalar.activation(zc[:], pb[:, 0:CW],
                                             AF.Sigmoid,
                                             bias=bias_sb[:, 1:2],
                                             scale=-1.0 / S)
                        # rh = r * (S*gh_n + S*b_hn)
                        rh = gate.tile([128, CW], F16, tag="rh")
                        nc.vector.scalar_tensor_tensor(
                            rh[:], pb[:, CW:2 * CW], bias_sb[:, 3:4], r[:],
                            op0=OP.add, op1=OP.mult)
                        # s = (S*gi_n + S*b_in) + rh
                        s = gate.tile([128, CW], F16, tag="s")
                        nc.vector.scalar_tensor_tensor(
                            s[:], pa[:, CW:2 * CW], bias_sb[:, 2:3], rh[:],
                            op0=OP.add, op1=OP.add)
                        nng = gate.tile([128, CW], BF16, tag="nng")
                        nc.scalar.activation(nng[:], s[:], AF.Tanh,
                                             scale=1.0 / S)
                        # h' = zc*n + (1-zc)*h = zc*n - (zc-1)*h
                        v2n = gate.tile([128, CW], BF16, tag="v2n")
                        nc.vector.scalar_tensor_tensor(
                            v2n[:], zc[:], 1.0, hprev[:],
                            op0=OP.subtract, op1=OP.mult)
                        u = gate.tile([128, CW], BF16, tag="u")
                        nc.vector.tensor_tensor(u[:], nng[:], zc[:],
                                                op=OP.mult)
                        nc.vector.tensor_tensor(hnew[:], u[:], v2n[:],
                                                op=OP.subtract)
                        uv_cur.append((u, v2n))
                    h_tiles[st] = hnew_c
                    uv_prev = uv_cur

                    j = LEAD + st - 1
                    if j < TS:
                        dma_msg(j)

            # ---- output head: fc1 + ELU, fc2 + sigmoid ----
            with tc.tile_pool(name="psf", bufs=2, space="PSUM") as psf:
                hlast = h_tiles[T - 1]
                hid = []
                for c in range(FC // 128):
                    pf = psf.tile([128, BS], F32, tag="pf")
                    for ch in range(NCH):
                        nc.tensor.matmul(pf[:, bass.ts(ch, CW)],
                                         wfc1_sb[:, bass.ts(c, 128)],
                                         hlast[ch][:],
                                         start=True, stop=True)
                    bcol = bias_sb[:, 5 + c:6 + c]
                    x1 = fcp.tile([128, BS], BF16, tag="fcx")
                    nc.vector.tensor_scalar_add(x1[:], pf[:], bcol)
                    e1 = fcp.tile([128, BS], F32, tag="fce")
                    nc.scalar.activation(e1[:], pf[:], AF.Exp, bias=bcol)
                    # elu(x) = max(x,0) + min(exp(x)-1, 0)
                    em = fcp.tile([128, BS], BF16, tag="fcm")
                    nc.vector.scalar_tensor_tensor(em[:], e1[:], -1.0,
                                                   zeros[:],
                                                   op0=OP.add, op1=OP.min)
                    hc = fcp.tile([128, BS], BF16, tag="hid" + str(c))
                    nc.vector.scalar_tensor_tensor(hc[:], x1[:], 0.0, em[:],
                                                   op0=OP.max, op1=OP.add)
                    hid.append(hc)
                for o in range(O // 128):
                    po = psf.tile([128, BS], F32, tag="po")
                    for c in range(FC // 128):
                        nc.tensor.matmul(po[:], wfc2_sb[:, c, bass.ts(o, 128)],
                                         hid[c][:], start=(c == 0),
                                         stop=(c == FC // 128 - 1))
                    ob = outp.tile([128, BS], F32)
                    nc.scalar.activation(ob[:], po[:], AF.Sigmoid,
                                         bias=bias_sb[:, 7 + o:8 + o])
                    nc.sync.dma_start(out[o], ob[:])

    _split_excess_waits(nc)
    return nc


def _sigmoid(x):
    return 1.0 / (1.0 + np.exp(-x))


def kernel(message, W_emb, b_emb, init_emb, W_ih, W_hh, b_ih, b_hh,
           W_fc1, b_fc1, W_fc2, b_fc2, _trace=False, _trace_kwargs=None):
    global _PROGRAM, LAST_RESULTS
    if _PROGRAM is None:
        _PROGRAM = _build_program()
    nc = _PROGRAM

    f32 = np.float32

    import ml_dtypes
    bf16 = ml_dtypes.bfloat16
    fp8 = ml_dtypes.float8_e4m3fn

    # message -> per-core fp8 DoubleRow layout [t, p, k, b]; the last token's
    # embedding is never consumed by the GRU so only t = 0..62 is shipped.
    msgT = (
        (message[:, :TS, :] * SM)
        .reshape(N_CORES, BS, TS, 4, 128)
        .transpose(0, 2, 4, 3, 1)  # [core, t, p, k, b]
        .astype(fp8)
    )
    msgT = np.ascontiguousarray(msgT)

    # fused input projection: gi = W_combo @ msg + (W_ih@b_emb + b_ih)
    W_combo = W_ih.astype(np.float64) @ W_emb.astype(np.float64)  # [3H, V]
    wcombo = np.ascontiguousarray(
        (SW * W_combo).T.reshape(4, 128, 3 * H).transpose(1, 0, 2)
    ).astype(fp8)

    whhT3 = (S * W_hh.astype(np.float64)).T  # [H, 3H]
    whhT = np.ascontiguousarray(
        np.concatenate([whhT3, -whhT3], axis=1)
    ).astype(bf16)
    wfc1T = np.ascontiguousarray(W_fc1.T).astype(bf16)
    wfc2T = np.ascontiguousarray(W_fc2.T).astype(bf16)

    b_combo = (W_ih.astype(np.float64) @ b_emb.astype(np.float64)
               + b_ih.astype(np.float64))
    b_hh64 = b_hh.astype(np.float64)

    # step 0 is batch-independent: h0 = 0, x0 = init_emb
    gi0 = W_ih.astype(np.float64) @ init_emb[0, 0].astype(np.float64) + b_ih
    r0 = _sigmoid(gi0[0:H] + b_hh64[0:H])
    z0 = _sigmoid(gi0[H:2 * H] + b_hh64[H:2 * H])
    n0 = np.tanh(gi0[2 * H:3 * H] + r0 * b_hh64[2 * H:3 * H])
    h1 = (1.0 - z0) * n0

    bias = np.zeros((128, 15), f32)
    bias[:, 0] = (b_combo + b_hh64)[0:H]
    bias[:, 1] = -(b_combo + b_hh64)[H:2 * H]
    bias[:, 2] = S * b_combo[2 * H:3 * H]
    bias[:, 3] = S * b_hh64[2 * H:3 * H]
    bias[:, 4] = h1
    bias[:, 5:7] = b_fc1.reshape(2, 128).T.astype(f32)
    bias[:, 7:15] = b_fc2.reshape(8, 128).T.astype(f32)

    shared = dict(wcombo=wcombo, whhT=whhT, wfc1T=wfc1T,
                  wfc2T=wfc2T, bias=bias)
    in_maps = [dict(msg=msgT[c], **shared) for c in range(N_CORES)]

    kw = dict(_trace_kwargs or {})
    res = run_bass_kernel_spmd(nc, in_maps, list(range(N_CORES)),
                               trace=_trace, **kw)
    LAST_RESULTS = res

    outs = [res.results[c]["out"].reshape(O, BS).T for c in range(N_CORES)]
    return np.ascontiguousarray(np.concatenate(outs, axis=0), dtype=f32)


# revision 29
# speedup vs baseline: 1.1060x; 1.0281x over previous
"""Trainium2 Bass kernel for the GRU decoder problem.

Math (reference):
    emb[b,t]   = W_emb @ message[b,t] + b_emb                  # [B,T,E]
    xs[t]      = init_emb (t=0) else emb[:, t-1]               # GRU inputs
    gi[t]      = W_ih @ xs[t] + b_ih                           # [B,3H]
    gh         = W_hh @ h + b_hh
    r          = sigmoid(gi_r + gh_r); z = sigmoid(gi_z + gh_z)
    n          = tanh(gi_n + r * gh_n)
    h'         = (1-z)*n + z*h
    out        = sigmoid(W_fc2 @ elu(W_fc1 @ h_T + b_fc1) + b_fc2)

Device strategy (pure data parallel over batch, 8 cores, B/core = 512):
  - The embedding and input-gate projections are fused on host into
    W_combo = W_ih @ W_emb [3H, V]; the per-step input-gate pre-activations
    are computed directly from the fp8 message with fp8 DoubleRow matmuls
    (2 K-subtile pairs instead of 4 bf16 K-chunks), eliminating the separate
    embedding matmul + PSUM eviction of the previous revision.
  - Scaling: message x32 and W_combo x64 keep fp8e4m3 values in the normal
    range; W_hh is pre-scaled by 2048 in bf16 so the PSUM pre-activations are
    uniformly S-scaled, and the 1/S is folded into the ACT scale operand of
    the sigmoid/tanh ops (biases ride along unscaled / pre-scaled).
  - Step 0 of the GRU is batch-independent (h0 = 0, x0 = init_emb), so h1 is
    computed on the host and broadcast; the device scan runs steps 1..63.
  - Batch is split into two 256-wide chains so one chain's serial
    recurrence path hides under the other's engine work. Input matmuls for
    step t+1 are emitted before the gh matmuls of step t so the in-order PE
    queue fills recurrence stalls with independent work.
"""

import numpy as np

import concourse.bass as bass
import concourse.tile as tile
import concourse.mybir as mybir
from concourse.bass_utils import run_bass_kernel_spmd

N_CORES = 8
B, T, V, E, H, FC, O = 4096, 64, 512, 32, 128, 256, 1024
BS = B // N_CORES      # batch per core
TS = T - 1             # message slices consumed by the GRU
NCH = 2                # batch chains per core
CW = BS // NCH         # chain width

SW = 64.0              # host scale on W_combo (fp8 range centering)
SM = 32.0              # host scale on message
S = SW * SM            # combined scale of PSUM pre-activations

F8 = mybir.dt.float8e4
F16 = mybir.dt.float16
BF16 = mybir.dt.bfloat16
F32 = mybir.dt.float32
AF = mybir.ActivationFunctionType
OP = mybir.AluOpType
DR = mybir.MatmulPerfMode.DoubleRow

_PROGRAM = None
LAST_RESULTS = None


# walrus codegen in this toolchain encodes at most 1 sem wait per
# instruction; excess waits are hoisted onto NoOp carriers.
_WAIT_LIMITS: dict = {}
_DEFAULT_WAIT_LIMIT = 1


def _split_excess_waits(nc):
    """Hoist sem waits beyond an instruction encoding's capacity onto
    preceding same-engine NoOp carriers (engines execute their queues in
    order, so waiting earlier on the same engine is equivalent)."""
    for f in nc.m.functions:
        for bb in f.blocks:
            newlist = []
            changed = False
            for inst in bb.instructions:
                si = inst.sync_info
                limit = _WAIT_LIMITS.get(type(inst).__name__, _DEFAULT_WAIT_LIMIT)
                if (
                    limit is not None
                    and si is not None
                    and si.on_wait is not None
                    and len(si.on_wait) > limit
                ):
                    waits = list(si.on_wait)
                    for k, w in enumerate(waits[:-limit]):
                        carrier = mybir.InstNoOp(
                            name=f"{inst.name}-wsplit{k}", ins=[], outs=[]
                        )
                        carrier.engine = inst.engine
                        carrier.sync_info = mybir.SyncInfo(on_wait=[w], on_update=[])
                        newlist.append(carrier)
                    si.on_wait = waits[-limit:]
                    inst.sync_info = si
                    changed = True
                newlist.append(inst)
            if changed:
                bb.instructions[:] = newlist


def _build_program():
    nc = bass.Bass()

    # msg[t, p, k, b] = SM * message[b, t, 128k + p]  (fp8 DoubleRow layout)
    msg = nc.dram_tensor("msg", [TS, 128, 4, BS], F8, kind="ExternalInput")
    # wcombo[p, k, j] = SW * (W_ih @ W_emb)[j, 128k + p]
    wcombod = nc.dram_tensor("wcombo", [128, 4, 3 * H], F8, kind="ExternalInput")
    # S-scaled; sections 3..5 are the negated gate blocks so every gh matmul
    # can be computed as whh@u + (-whh)@v2n from the pre-combine tensors
    # (h' = u - v2n), taking the h' add off the recurrence matmul path.
    whhT = nc.dram_tensor("whhT", [H, 6 * H], BF16, kind="ExternalInput")
    wfc1T = nc.dram_tensor("wfc1T", [H, FC], BF16, kind="ExternalInput")
    wfc2T = nc.dram_tensor("wfc2T", [FC, O], BF16, kind="ExternalInput")
    # bias columns: 0 r, 1 -(z), 2 S*b_in, 3 S*b_hn, 4 h1, 5..6 fc1, 7..14 fc2
    biasd = nc.dram_tensor("bias", [128, 15], F32, kind="ExternalInput")
    out = nc.dram_tensor("out", [O // 128, 128, BS], F32, kind="ExternalOutput")

    with tile.TileContext(nc) as tc:
        with (
            tc.tile_pool(name="const", bufs=1) as const,
            tc.tile_pool(name="msgp", bufs=8) as msgp,
            tc.tile_pool(name="gate", bufs=6) as gate,
            tc.tile_pool(name="fcp", bufs=2) as fcp,
            tc.tile_pool(name="hp", bufs=3) as hp,
            tc.tile_pool(name="outp", bufs=2) as outp,
        ):
            # ---- resident constants ----
            wcombo_sb = const.tile([128, 4, 3 * H], F8)
            nc.sync.dma_start(wcombo_sb[:], wcombod[:])
            whh_sb = const.tile([H, 6 * H], BF16)
            nc.sync.dma_start(whh_sb[:], whhT[:])
            # fc weights ride the (otherwise idle) gpsimd DMA queue so the
            # big wfc2 transfer doesn't delay the message prologue.
            wfc1_sb = const.tile([H, FC], BF16)
            nc.gpsimd.dma_start(wfc1_sb[:], wfc1T[:])
            wfc2_sb = const.tile([128, FC // 128, O], BF16)
            nc.gpsimd.dma_start(wfc2_sb[:],
                                wfc2T.rearrange("(c p) o -> p c o", p=128))
            bias_sb = const.tile([128, 15], F32)
            nc.sync.dma_start(bias_sb[:], biasd[:])
            zeros = const.tile([128, BS], BF16)
            nc.gpsimd.memset(zeros[:], 0.0)

            # h after step 0 is batch-independent (host-computed) -> broadcast.
            # One tile per chain so the chains' recurrences stay decoupled in
            # Tile's dependency tracking.
            h_tiles = {}
            h0c = []
            for c in range(NCH):
                hc = hp.tile([H, CW], BF16, tag="h" + str(c))
                nc.vector.tensor_scalar_add(hc[:], zeros[:, 0:CW],
                                            bias_sb[:, 4:5])
                h0c.append(hc)
            h_tiles[0] = h0c

            # ---- message DMA prologue ----
            LEAD = 6
            msg_tiles = {}

            def dma_msg(j):
                mt = msgp.tile([128, 4, BS], F8)
                nc.sync.dma_start(mt[:], msg[j])
                msg_tiles[j] = mt

            for j in range(min(LEAD, TS)):
                dma_msg(j)

            # Input-gate pre-activations for step st (consumes msg slice
            # st-1): fp8 DoubleRow matmuls, 2 K-subtile pairs per gate, per
            # chain. PSUM dependencies are tile(bank)-granular, so banks are
            # grouped by when their LAST writer lands: pa = [r | gh_n] (all
            # writers gated only on u/v2n), pb = [z | gi_n].
            pa_tiles = {}
            pb_tiles = {}

            with tc.tile_pool(name="psg", bufs=2, space="PSUM") as psg:

                def emit_input_mms(st):
                    mt = msg_tiles[st - 1]
                    pas, pbs = [], []
                    for c in range(NCH):
                        sl = bass.ts(c, CW)
                        pa = psg.tile([128, 2 * CW], F32, tag="pa" + str(c))
                        pb = psg.tile([128, 2 * CW], F32, tag="pb" + str(c))
                        # pa = [r | z], pb = [gi_n | gh_n].
                        for dst, g, sp_ in (
                            (pa[:, 0:CW], 0, False),
                            (pa[:, CW:2 * CW], 1, False),
                            (pb[:, 0:CW], 2, True),
                        ):
                            for kk in (0, 2):
                                nc.tensor.matmul(
                                    dst,
                                    wcombo_sb[:, kk:kk + 2, bass.ts(g, H)],
                                    mt[:, kk:kk + 2, sl],
                                    start=(kk == 0),
                                    stop=(sp_ and kk == 2),
                                    perf_mode=DR)
                        pas.append(pa)
                        pbs.append(pb)
                    pa_tiles[st] = pas
                    pb_tiles[st] = pbs

                emit_input_mms(1)

                for st in range(1, T):
                    # Input matmuls for step st+1 go to the PE queue before
                    # the gh matmuls of step st: the in-order PE works on
                    # them while waiting for h[st-1].
                    if st + 1 <= T - 1:
                        emit_input_mms(st + 1)

                    hnew_c = []
                    uv_cur = []
                    for c in range(NCH):
                        pa = pa_tiles[st][c]
                        pb = pb_tiles[st][c]
                        hprev = h_tiles[st - 1][c]
                        hnew = hp.tile([H, CW], BF16, tag="h" + str(c))
                        hnew_c.append(hnew)
                        # gh_r comes from u/v2n (h' = u - v2n) so the r path
                        # skips the h' add; gh_n/gh_z read h' but land in pb,
                        # whose consumers run later.
                        if st == 1:
                            nc.tensor.matmul(pa[:, 0:CW], whh_sb[:, 0:H],
                                             hprev[:], start=False, stop=True)
                        else:
                            up, vp = uv_prev[c]
                            nc.tensor.matmul(pa[:, 0:CW], whh_sb[:, 0:H],
                                             up[:], start=False, stop=False)
                            nc.tensor.matmul(pa[:, 0:CW],
                                             whh_sb[:, 3 * H:4 * H], vp[:],
                                             start=False, stop=True)
                        nc.tensor.matmul(pb[:, CW:2 * CW],
                                         whh_sb[:, 2 * H:3 * H],
                                         hprev[:], start=True, stop=True)
                        nc.tensor.matmul(pa[:, CW:2 * CW], whh_sb[:, H:2 * H],
                                         hprev[:], start=False, stop=True)

                        r = gate.tile([128, CW], F16, tag="r")
                        nc.scalar.activation(r[:], pa[:, 0:CW], AF.Sigmoid,
                                             bias=bias_sb[:, 0:1],
                                             scale=1.0 / S)
                        # zc = 1 - z = sigmoid(-(x/S + bz)); bias col 1 = -bz
                        zc = gate.tile([128, CW], BF16, tag="zc")
                        nc.scalar.activation(zc[:], pa[:, CW:2 * CW],
                                             AF.Sigmoid,
                                             bias=bias_sb[:, 1:2],
                                             scale=-1.0 / S)
                        # rh = r * (S*gh_n + S*b_hn)
                        rh = gate.tile([128, CW], F16, tag="rh")
                        nc.vector.scalar_tensor_tensor(
                            rh[:], pb[:, CW:2 * CW], bias_sb[:, 3:4], r[:],
                            op0=OP.add, op1=OP.mult)
                        # s = (S*gi_n + S*b_in) + rh
                        s = gate.tile([128, CW], F16, tag="s")
                        nc.vector.scalar_tensor_tensor(
                            s[:], pb[:, 0:CW], bias_sb[:, 2:3], rh[:],
                            op0=OP.add, op1=OP.add)
                        nng = gate.tile([128, CW], BF16, tag="nng")
                        nc.scalar.activation(nng[:], s[:], AF.Tanh,
                                             scale=1.0 / S)
                        # h' = zc*n + (1-zc)*h = zc*n - (zc-1)*h
                        v2n = gate.tile([128, CW], BF16, tag="v2n")
                        nc.vector.scalar_tensor_tensor(
                            v2n[:], zc[:], 1.0, hprev[:],
                            op0=OP.subtract, op1=OP.mult)
                        u = gate.tile([128, CW], BF16, tag="u")
                        nc.vector.tensor_tensor(u[:], nng[:], zc[:],
                                                op=OP.mult)
                        nc.vector.tensor_tensor(hnew[:], u[:], v2n[:],
                                                op=OP.subtract)
                        uv_cur.append((u, v2n))
                    h_tiles[st] = hnew_c
                    uv_prev = uv_cur

                    j = LEAD + st - 1
                    if j < TS:
                        dma_msg(j)

            # ---- output head: fc1 + ELU, fc2 + sigmoid ----
            with tc.tile_pool(name="psf", bufs=2, space="PSUM") as psf:
                hlast = h_tiles[T - 1]
                hid = []
                for c in range(FC // 128):
                    pf = psf.tile([128, BS], F32, tag="pf")
                    for ch in range(NCH):
                        nc.tensor.matmul(pf[:, bass.ts(ch, CW)],
                                         wfc1_sb[:, bass.ts(c, 128)],
                                         hlast[ch][:],
                                         start=True, stop=True)
                    bcol = bias_sb[:, 5 + c:6 + c]
                    x1 = fcp.tile([128, BS], BF16, tag="fcx")
                    nc.vector.tensor_scalar_add(x1[:], pf[:], bcol)
                    e1 = fcp.tile([128, BS], F32, tag="fce")
                    nc.scalar.activation(e1[:], pf[:], AF.Exp, bias=bcol)
                    # elu(x) = max(x,0) + min(exp(x)-1, 0)
                    em = fcp.tile([128, BS], BF16, tag="fcm")
                    nc.vector.scalar_tensor_tensor(em[:], e1[:], -1.0,
                                                   zeros[:],
                                                   op0=OP.add, op1=OP.min)
                    hc = fcp.tile([128, BS], BF16, tag="hid" + str(c))
                    nc.vector.scalar_tensor_tensor(hc[:], x1[:], 0.0, em[:],
                                                   op0=OP.max, op1=OP.add)
                    hid.append(hc)
                for o in range(O // 128):
                    po = psf.tile([128, BS], F32, tag="po")
                    for c in range(FC // 128):
                        nc.tensor.matmul(po[:], wfc2_sb[:, c, bass.ts(o, 128)],
                                         hid[c][:], start=(c == 0),
                                         stop=(c == FC // 128 - 1))
                    ob = outp.tile([128, BS], F32)
                    nc.scalar.activation(ob[:], po[:], AF.Sigmoid,
                                         bias=bias_sb[:, 7 + o:8 + o])
                    nc.sync.dma_start(out[o], ob[:])

    _split_excess_waits(nc)
    return nc


def _sigmoid(x):
    return 1.0 / (1.0 + np.exp(-x))


def kernel(message, W_emb, b_emb, init_emb, W_ih, W_hh, b_ih, b_hh,
           W_fc1, b_fc1, W_fc2, b_fc2, _trace=False, _trace_kwargs=None):
    global _PROGRAM, LAST_RESULTS
    if _PROGRAM is None:
        _PROGRAM = _build_program()
    nc = _PROGRAM

    f32 = np.float32

    import ml_dtypes
    bf16 = ml_dtypes.bfloat16
    fp8 = ml_dtypes.float8_e4m3fn

    # message -> per-core fp8 DoubleRow layout [t, p, k, b]; the last token's
    # embedding is never consumed by the GRU so only t = 0..62 is shipped.
    msgT = (
        (message[:, :TS, :] * SM)
        .reshape(N_CORES, BS, TS, 4, 128)
        .transpose(0, 2, 4, 3, 1)  # [core, t, p, k, b]
        .astype(fp8)
    )
    msgT = np.ascontiguousarray(msgT)

    # fused input projection: gi = W_combo @ msg + (W_ih@b_emb + b_ih)
    W_combo = W_ih.astype(np.float64) @ W_emb.astype(np.float64)  # [3H, V]
    wcombo = np.ascontiguousarray(
        (SW * W_combo).T.reshape(4, 128, 3 * H).transpose(1, 0, 2)
    ).astype(fp8)

    whhT3 = (S * W_hh.astype(np.float64)).T  # [H, 3H]
    whhT = np.ascontiguousarray(
        np.concatenate([whhT3, -whhT3], axis=1)
    ).astype(bf16)
    wfc1T = np.ascontiguousarray(W_fc1.T).astype(bf16)
    wfc2T = np.ascontiguousarray(W_fc2.T).astype(bf16)

    b_combo = (W_ih.astype(np.float64) @ b_emb.astype(np.float64)
               + b_ih.astype(np.float64))
    b_hh64 = b_hh.astype(np.float64)

    # step 0 is batch-independent: h0 = 0, x0 = init_emb
    gi0 = W_ih.astype(np.float64) @ init_emb[0, 0].astype(np.float64) + b_ih
    r0 = _sigmoid(gi0[0:H] + b_hh64[0:H])
    z0 = _sigmoid(gi0[H:2 * H] + b_hh64[H:2 * H])
    n0 = np.tanh(gi0[2 * H:3 * H] + r0 * b_hh64[2 * H:3 * H])
    h1 = (1.0 - z0) * n0

    bias = np.zeros((128, 15), f32)
    bias[:, 0] = (b_combo + b_hh64)[0:H]
    bias[:, 1] = -(b_combo + b_hh64)[H:2 * H]
    bias[:, 2] = S * b_combo[2 * H:3 * H]
    bias[:, 3] = S * b_hh64[2 * H:3 * H]
    bias[:, 4] = h1
    bias[:, 5:7] = b_fc1.reshape(2, 128).T.astype(f32)
    bias[:, 7:15] = b_fc2.reshape(8, 128).T.astype(f32)

    shared = dict(wcombo=wcombo, whhT=whhT, wfc1T=wfc1T,
                  wfc2T=wfc2T, bias=bias)
    in_maps = [dict(msg=msgT[c], **shared) for c in range(N_CORES)]

    kw = dict(_trace_kwargs or {})
    res = run_bass_kernel_spmd(nc, in_maps, list(range(N_CORES)),
                               trace=_trace, **kw)
    LAST_RESULTS = res

    outs = [res.results[c]["out"].reshape(O, BS).T for c in range(N_CORES)]
    return np.ascontiguousarray(np.concatenate(outs, axis=0), dtype=f32)
